# revision 9
# baseline (speedup 1.0000x reference)
"""Trainium2 Bass kernel for nn_BroadcastTC.

Math (per (b, c) pair; T1/T2 are 27-element tensors viewed as 3x3x3):
  out1[b,c,x,y,u,v] = sum_k T1[b,c,x,y,k] * T2[b,c,k,u,v] / sqrt(3)
  out2[b,c,x,u]     = sum_{k,l} T1[b,c,x,k,l] * T2[b,c,k,l,u] / 3
  out3[b,c]         = sum_{k,l,m} T1[b,c,k,l,m] * T2[b,c,k,l,m] / sqrt(27)

Pure data parallel over 8 NeuronCores (batch axis). Each core processes its
131072 (b,c)-rows in tiles of [128 partitions x R rows].

v3 engine split (DVE and GPSIMD serialize on the shared SBUF port, so GPSIMD
is retired; the independent-port engines PE/ACT take the accumulations):
  - VectorE:  all elementwise multiplies (out1: 3 broadcast mults; out2:
              3 x-split mults; out3: dense mult) + out2 add-tree + out3 reduce
  - TensorE:  out1 = P0+P1+P2 via exact fp32 identity matmuls accumulated
              in PSUM (bank-aligned 432-col chunks)
  - ScalarE:  T1 pre-scales (normalizers fold in), PSUM->SBUF drain of out1,
              out3 post-scale
  - All DMA on HWDGE (nc.sync).
"""

import sys

import numpy as np

B, C = 8192, 128
N_CORES = 8
P = 128            # SBUF partitions
R = 32             # rows per partition per tile
C27 = 27
ROWS = B * C                      # 1048576 total (b,c) rows
RPC = ROWS // N_CORES             # 131072 rows per core
TILE_ROWS = P * R                 # 4096 rows per tile
NT = RPC // TILE_ROWS             # 32 tiles per core

HALF = R * 81 // 2                # 1296 out1 cols per half-tile
CHUNK = HALF // 3                 # 432 cols per PSUM bank (<=512)

_C1 = float(1.0 / np.sqrt(3.0))   # out1 normalizer (folded into PE weights)
_C2 = float(1.0 / 3.0)            # out2 normalizer (ACT post-scale)
_C3 = float(1.0 / np.sqrt(27.0))  # out3 normalizer (ACT post-scale)

_CACHE = {}


def _ensure_paths():
    for p in ("/opt/trn_rl_repo", "/root/.axon_site/_ro/trn_rl_repo"):
        try:
            import concourse  # noqa: F401
            return
        except ImportError:
            if p not in sys.path:
                sys.path.insert(0, p)
    import concourse  # noqa: F401


def _build_nc():
    import concourse.bacc as bacc
    import concourse.tile as tile
    from concourse import mybir

    DT = mybir.dt.float32
    add = mybir.AluOpType.add

    nc = bacc.Bacc("TRN2", target_bir_lowering=False, debug=False)

    t1_d = nc.dram_tensor("t1", [NT, P, R * C27], DT, kind="ExternalInput")
    t2_d = nc.dram_tensor("t2", [NT, P, R * C27], DT, kind="ExternalInput")
    o1_d = nc.dram_tensor("o1", [NT, P, R * 81], DT, kind="ExternalOutput")
    o2_d = nc.dram_tensor("o2", [NT, P, R * 9], DT, kind="ExternalOutput")
    o3_d = nc.dram_tensor("o3", [NT, P, R], DT, kind="ExternalOutput")

    ident_dram = nc.inline_tensor(
        (_C1 * np.eye(P)).astype(np.float32), name="ident"
    )
    ident1_dram = nc.inline_tensor(np.eye(P, dtype=np.float32), name="ident1")

    with tile.TileContext(nc) as tc:
        with tc.tile_pool(name="pool", bufs=1) as pool, tc.tile_pool(
            name="psum", bufs=2, space="PSUM"
        ) as psum_pool:
            ident = pool.tile([P, P], DT, name="ident", bufs=1)
            nc.sync.dma_start(out=ident, in_=ident_dram.ap())
            ident1 = pool.tile([P, P], DT, name="ident1", bufs=1)
            nc.sync.dma_start(out=ident1, in_=ident1_dram.ap())

            for t in range(NT):
                t1 = pool.tile([P, R * C27], DT, name="t1t", bufs=4)
                t2 = pool.tile([P, R * C27], DT, name="t2t", bufs=4)
                nc.sync.dma_start(out=t1, in_=t1_d[t])
                nc.sync.dma_start(out=t2, in_=t2_d[t])

                t1v = t1.rearrange("p (r c) -> p r c", c=C27)
                t2v = t2.rearrange("p (r c) -> p r c", c=C27)

                # ---- out1: DVE broadcast mults -> PE accumulate -> ACT drain
                pb = pool.tile([P, 3, R * 81], DT, name="pb", bufs=2)
                for k in range(3):
                    a = t1v.rearrange("p r (x k) -> p r x k", k=3)[:, :, :, k]
                    a = a.unsqueeze(3).broadcast_to([P, R, 9, 9])
                    b = t2v.rearrange("p r (k u) -> p r k u", k=3)[:, :, k, :]
                    b = b.unsqueeze(2).broadcast_to([P, R, 9, 9])
                    nc.vector.tensor_mul(
                        pb[:, k].rearrange("p (r x u) -> p r x u", x=9, u=9), a, b
                    )
                o1 = pool.tile([P, R * 81], DT, name="o1", bufs=3)
                for h in range(2):
                    acc = psum_pool.tile([P, 3, 512], DT, name="acc")
                    for c in range(3):
                        lo = h * HALF + c * CHUNK
                        for k in range(3):
                            nc.tensor.matmul(
                                acc[:, c, :CHUNK],
                                ident,
                                pb[:, k, lo : lo + CHUNK],
                                start=(k == 0),
                                stop=(k == 2),
                            )
                    nc.scalar.copy(
                        o1[:, h * HALF : (h + 1) * HALF], acc[:, :, :CHUNK]
                    )

                # ---- out2 on DVE (x-split mults + add tree) ----
                # qq[r, x, u, m] = t1b[r, 9x+m] * t2[r, 3m+u]
                qq = pool.tile([P, R * 81], DT, name="qq", bufs=2)
                qqx = qq.rearrange("p (r x u m) -> p r x u m", x=3, u=3, m=9)
                for x in range(3):
                    a2 = t1v.rearrange("p r (x m) -> p r x m", x=3)[:, :, x, :]
                    a2 = a2.unsqueeze(2).broadcast_to([P, R, 3, 9])  # (R, u, m)
                    b2 = t2v.rearrange("p r (m u) -> p r m u", u=3)
                    b2 = b2.transpose([0, 1, 3, 2])  # (R, u, m) strides (27,1,3)
                    nc.vector.tensor_mul(qqx[:, :, x, :, :], a2, b2)
                qqv = qq.rearrange("p (r xu m) -> p r xu m", xu=9, m=9)
                # PE partial-sum over m=0..3 into PSUM; DVE reduces m=4..8
                acc2 = psum_pool.tile([P, 512], DT, name="acc2")
                for m in range(4):
                    nc.tensor.matmul(
                        acc2[:, : R * 9],
                        ident1,
                        qqv[:, :, :, m],
                        start=(m == 0),
                        stop=(m == 3),
                    )
                o2p = pool.tile([P, R * 9], DT, name="o2p", bufs=2)
                nc.vector.tensor_reduce(
                    o2p.rearrange("p (r xu) -> p r xu", xu=9),
                    qqv[:, :, :, 4:9],
                    mybir.AxisListType.X,
                    add,
                )
                o2r = pool.tile([P, R * 9], DT, name="o2r", bufs=2)
                nc.vector.tensor_add(o2r, o2p, acc2[:, : R * 9])
                o2 = pool.tile([P, R * 9], DT, name="o2", bufs=3)
                nc.scalar.mul(o2, o2r, _C2)

                # -------- out3: DVE mult + DVE reduce + ACT scale --------
                e = pool.tile([P, R * C27], DT, name="e", bufs=2)
                nc.vector.tensor_mul(e, t1, t2)
                o3r = pool.tile([P, R], DT, name="o3r", bufs=2)
                nc.vector.tensor_reduce(
                    o3r,
                    e.rearrange("p (r c) -> p r c", c=C27),
                    mybir.AxisListType.X,
                    add,
                )
                o3 = pool.tile([P, R], DT, name="o3", bufs=3)
                nc.scalar.mul(o3, o3r, _C3)

                nc.sync.dma_start(out=o1_d[t], in_=o1)
                nc.sync.dma_start(out=o2_d[t], in_=o2)
                nc.sync.dma_start(out=o3_d[t], in_=o3)

    nc.compile()
    return nc


def _get_nc():
    if "nc" not in _CACHE:
        _ensure_paths()
        _CACHE["nc"] = _build_nc()
    return _CACHE["nc"]


def kernel(T1, T2, _trace=False):
    _ensure_paths()
    from concourse.bass_utils import run_bass_kernel_spmd

    nc = _get_nc()

    T1f = np.ascontiguousarray(np.asarray(T1, dtype=np.float32)).reshape(ROWS, C27)
    T2f = np.ascontiguousarray(np.asarray(T2, dtype=np.float32)).reshape(ROWS, C27)

    in_maps = []
    for i in range(N_CORES):
        sl = slice(i * RPC, (i + 1) * RPC)
        in_maps.append(
            {
                "t1": np.ascontiguousarray(T1f[sl]).reshape(NT, P, R * C27),
                "t2": np.ascontiguousarray(T2f[sl]).reshape(NT, P, R * C27),
            }
        )

    res = run_bass_kernel_spmd(nc, in_maps, list(range(N_CORES)), trace=_trace)
    _CACHE["last_results"] = res

    out1 = np.empty((ROWS, 81), dtype=np.float32)
    out2 = np.empty((ROWS, 9), dtype=np.float32)
    out3 = np.empty((ROWS,), dtype=np.float32)
    for i in range(N_CORES):
        sl = slice(i * RPC, (i + 1) * RPC)
        r = res.results[i]
        out1[sl] = r["o1"].reshape(RPC, 81)
        out2[sl] = r["o2"].reshape(RPC, 9)
        out3[sl] = r["o3"].reshape(RPC)

    return (
        out1.reshape(B, C, 3, 3, 3, 3),
        out2.reshape(B, C, 3, 3),
        out3.reshape(B, C),
    )


# revision 10
# speedup vs baseline: 1.0418x; 1.0418x over previous
"""Trainium2 Bass kernel for nn_BroadcastTC.

Math (per (b, c) pair; T1/T2 are 27-element tensors viewed as 3x3x3):
  out1[b,c,x,y,u,v] = sum_k T1[b,c,x,y,k] * T2[b,c,k,u,v] / sqrt(3)
  out2[b,c,x,u]     = sum_{k,l} T1[b,c,x,k,l] * T2[b,c,k,l,u] / 3
  out3[b,c]         = sum_{k,l,m} T1[b,c,k,l,m] * T2[b,c,k,l,m] / sqrt(27)

Pure data parallel over 8 NeuronCores (batch axis). Each core processes its
131072 (b,c)-rows in tiles of [128 partitions x R rows].

v3 engine split (DVE and GPSIMD serialize on the shared SBUF port, so GPSIMD
is retired; the independent-port engines PE/ACT take the accumulations):
  - VectorE:  all elementwise multiplies (out1: 3 broadcast mults; out2:
              3 x-split mults; out3: dense mult) + out2 add-tree + out3 reduce
  - TensorE:  out1 = P0+P1+P2 via exact fp32 identity matmuls accumulated
              in PSUM (bank-aligned 432-col chunks)
  - ScalarE:  T1 pre-scales (normalizers fold in), PSUM->SBUF drain of out1,
              out3 post-scale
  - All DMA on HWDGE (nc.sync).
"""

import sys

import numpy as np

B, C = 8192, 128
N_CORES = 8
P = 128            # SBUF partitions
R = 32             # rows per partition per tile
C27 = 27
ROWS = B * C                      # 1048576 total (b,c) rows
RPC = ROWS // N_CORES             # 131072 rows per core
TILE_ROWS = P * R                 # 4096 rows per tile
NT = RPC // TILE_ROWS             # 32 tiles per core

HALF = R * 81 // 2                # 1296 out1 cols per half-tile
CHUNK = HALF // 3                 # 432 cols per PSUM bank (<=512)

_C1 = float(1.0 / np.sqrt(3.0))   # out1 normalizer (folded into PE weights)
_C2 = float(1.0 / 3.0)            # out2 normalizer (ACT post-scale)
_C3 = float(1.0 / np.sqrt(27.0))  # out3 normalizer (ACT post-scale)

_CACHE = {}


def _ensure_paths():
    for p in ("/opt/trn_rl_repo", "/root/.axon_site/_ro/trn_rl_repo"):
        try:
            import concourse  # noqa: F401
            return
        except ImportError:
            if p not in sys.path:
                sys.path.insert(0, p)
    import concourse  # noqa: F401


def _build_nc():
    import concourse.bacc as bacc
    import concourse.tile as tile
    from concourse import mybir

    DT = mybir.dt.float32
    add = mybir.AluOpType.add

    nc = bacc.Bacc("TRN2", target_bir_lowering=False, debug=False)

    t1_d = nc.dram_tensor("t1", [NT, P, R * C27], DT, kind="ExternalInput")
    t2_d = nc.dram_tensor("t2", [NT, P, R * C27], DT, kind="ExternalInput")
    o1_d = nc.dram_tensor("o1", [NT, P, R * 81], DT, kind="ExternalOutput")
    o2_d = nc.dram_tensor("o2", [NT, P, R * 9], DT, kind="ExternalOutput")
    o3_d = nc.dram_tensor("o3", [NT, P, R], DT, kind="ExternalOutput")

    ident_dram = nc.inline_tensor(
        (_C1 * np.eye(P)).astype(np.float32), name="ident"
    )

    with tile.TileContext(nc) as tc:
        with tc.tile_pool(name="pool", bufs=1) as pool, tc.tile_pool(
            name="psum", bufs=2, space="PSUM"
        ) as psum_pool:
            ident = pool.tile([P, P], DT, name="ident", bufs=1)
            nc.sync.dma_start(out=ident, in_=ident_dram.ap())

            for t in range(NT):
                t1 = pool.tile([P, R * C27], DT, name="t1t", bufs=4)
                t2 = pool.tile([P, R * C27], DT, name="t2t", bufs=4)
                nc.sync.dma_start(out=t1, in_=t1_d[t])
                nc.sync.dma_start(out=t2, in_=t2_d[t])

                t1v = t1.rearrange("p (r c) -> p r c", c=C27)
                t2v = t2.rearrange("p (r c) -> p r c", c=C27)

                # ---- out1: DVE broadcast mults -> PE accumulate -> ACT drain
                pb = pool.tile([P, 3, R * 81], DT, name="pb", bufs=2)
                for k in range(3):
                    a = t1v.rearrange("p r (x k) -> p r x k", k=3)[:, :, :, k]
                    a = a.unsqueeze(3).broadcast_to([P, R, 9, 9])
                    b = t2v.rearrange("p r (k u) -> p r k u", k=3)[:, :, k, :]
                    b = b.unsqueeze(2).broadcast_to([P, R, 9, 9])
                    nc.vector.tensor_mul(
                        pb[:, k].rearrange("p (r x u) -> p r x u", x=9, u=9), a, b
                    )
                o1 = pool.tile([P, R * 81], DT, name="o1", bufs=3)
                for h in range(2):
                    acc = psum_pool.tile([P, 3, 512], DT, name="acc")
                    for c in range(3):
                        lo = h * HALF + c * CHUNK
                        for k in range(3):
                            nc.tensor.matmul(
                                acc[:, c, :CHUNK],
                                ident,
                                pb[:, k, lo : lo + CHUNK],
                                start=(k == 0),
                                stop=(k == 2),
                            )
                    nc.scalar.copy(
                        o1[:, h * HALF : (h + 1) * HALF], acc[:, :, :CHUNK]
                    )

                # ---- out2 on DVE (x-split mults + add tree) ----
                # qq[r, x, u, m] = t1b[r, 9x+m] * t2[r, 3m+u]
                qq = pool.tile([P, R * 81], DT, name="qq", bufs=2)
                qqx = qq.rearrange("p (r x u m) -> p r x u m", x=3, u=3, m=9)
                for x in range(3):
                    a2 = t1v.rearrange("p r (x m) -> p r x m", x=3)[:, :, x, :]
                    a2 = a2.unsqueeze(2).broadcast_to([P, R, 3, 9])  # (R, u, m)
                    b2 = t2v.rearrange("p r (m u) -> p r m u", u=3)
                    b2 = b2.transpose([0, 1, 3, 2])  # (R, u, m) strides (27,1,3)
                    nc.vector.tensor_mul(qqx[:, :, x, :, :], a2, b2)
                qqv = qq.rearrange("p (r xu m) -> p r xu m", xu=9, m=9)
                o2r = pool.tile([P, R * 9], DT, name="o2r", bufs=2)
                nc.vector.tensor_reduce(
                    o2r.rearrange("p (r xu) -> p r xu", xu=9),
                    qqv,
                    mybir.AxisListType.X,
                    add,
                )
                o2 = pool.tile([P, R * 9], DT, name="o2", bufs=3)
                nc.scalar.mul(o2, o2r, _C2)

                # -------- out3: DVE mult + DVE reduce + ACT scale --------
                e = pool.tile([P, R * C27], DT, name="e", bufs=2)
                nc.vector.tensor_mul(e, t1, t2)
                o3r = pool.tile([P, R], DT, name="o3r", bufs=2)
                nc.vector.tensor_reduce(
                    o3r,
                    e.rearrange("p (r c) -> p r c", c=C27),
                    mybir.AxisListType.X,
                    add,
                )
                o3 = pool.tile([P, R], DT, name="o3", bufs=3)
                nc.scalar.mul(o3, o3r, _C3)

                nc.sync.dma_start(out=o1_d[t], in_=o1)
                nc.sync.dma_start(out=o2_d[t], in_=o2)
                nc.sync.dma_start(out=o3_d[t], in_=o3)

    nc.compile()
    return nc


def _get_nc():
    if "nc" not in _CACHE:
        _ensure_paths()
        _CACHE["nc"] = _build_nc()
    return _CACHE["nc"]


def kernel(T1, T2, _trace=False):
    _ensure_paths()
    from concourse.bass_utils import run_bass_kernel_spmd

    nc = _get_nc()

    T1f = np.ascontiguousarray(np.asarray(T1, dtype=np.float32)).reshape(ROWS, C27)
    T2f = np.ascontiguousarray(np.asarray(T2, dtype=np.float32)).reshape(ROWS, C27)

    in_maps = []
    for i in range(N_CORES):
        sl = slice(i * RPC, (i + 1) * RPC)
        in_maps.append(
            {
                "t1": np.ascontiguousarray(T1f[sl]).reshape(NT, P, R * C27),
                "t2": np.ascontiguousarray(T2f[sl]).reshape(NT, P, R * C27),
            }
        )

    res = run_bass_kernel_spmd(nc, in_maps, list(range(N_CORES)), trace=_trace)
    _CACHE["last_results"] = res

    out1 = np.empty((ROWS, 81), dtype=np.float32)
    out2 = np.empty((ROWS, 9), dtype=np.float32)
    out3 = np.empty((ROWS,), dtype=np.float32)
    for i in range(N_CORES):
        sl = slice(i * RPC, (i + 1) * RPC)
        r = res.results[i]
        out1[sl] = r["o1"].reshape(RPC, 81)
        out2[sl] = r["o2"].reshape(RPC, 9)
        out3[sl] = r["o3"].reshape(RPC)

    return (
        out1.reshape(B, C, 3, 3, 3, 3),
        out2.reshape(B, C, 3, 3),
        out3.reshape(B, C),
    )


# revision 12
# speedup vs baseline: 1.0444x; 1.0026x over previous
"""Trainium2 Bass kernel for nn_BroadcastTC.

Math (per (b, c) pair; T1/T2 are 27-element tensors viewed as 3x3x3):
  out1[b,c,x,y,u,v] = sum_k T1[b,c,x,y,k] * T2[b,c,k,u,v] / sqrt(3)
  out2[b,c,x,u]     = sum_{k,l} T1[b,c,x,k,l] * T2[b,c,k,l,u] / 3
  out3[b,c]         = sum_{k,l,m} T1[b,c,k,l,m] * T2[b,c,k,l,m] / sqrt(27)

Pure data parallel over 8 NeuronCores (batch axis). Each core processes its
131072 (b,c)-rows in tiles of [128 partitions x R rows].

v3 engine split (DVE and GPSIMD serialize on the shared SBUF port, so GPSIMD
is retired; the independent-port engines PE/ACT take the accumulations):
  - VectorE:  all elementwise multiplies (out1: 3 broadcast mults; out2:
              3 x-split mults; out3: dense mult) + out2 add-tree + out3 reduce
  - TensorE:  out1 = P0+P1+P2 via exact fp32 identity matmuls accumulated
              in PSUM (bank-aligned 432-col chunks)
  - ScalarE:  T1 pre-scales (normalizers fold in), PSUM->SBUF drain of out1,
              out3 post-scale
  - All DMA on HWDGE (nc.sync).
"""

import sys

import numpy as np

B, C = 8192, 128
N_CORES = 8
P = 128            # SBUF partitions
R = 32             # rows per partition per tile
C27 = 27
ROWS = B * C                      # 1048576 total (b,c) rows
RPC = ROWS // N_CORES             # 131072 rows per core
TILE_ROWS = P * R                 # 4096 rows per tile
NT = RPC // TILE_ROWS             # 32 tiles per core

HALF = R * 81 // 2                # 1296 out1 cols per half-tile
CHUNK = HALF // 3                 # 432 cols per PSUM bank (<=512)

_C1 = float(1.0 / np.sqrt(3.0))   # out1 normalizer (folded into PE weights)
_C2 = float(1.0 / 3.0)            # out2 normalizer (ACT post-scale)
_C3 = float(1.0 / np.sqrt(27.0))  # out3 normalizer (ACT post-scale)

_CACHE = {}


def _ensure_paths():
    for p in ("/opt/trn_rl_repo", "/root/.axon_site/_ro/trn_rl_repo"):
        try:
            import concourse  # noqa: F401
            return
        except ImportError:
            if p not in sys.path:
                sys.path.insert(0, p)
    import concourse  # noqa: F401


def _build_nc():
    import concourse.bacc as bacc
    import concourse.tile as tile
    from concourse import mybir

    DT = mybir.dt.float32
    add = mybir.AluOpType.add

    nc = bacc.Bacc("TRN2", target_bir_lowering=False, debug=False)

    t1_d = nc.dram_tensor("t1", [NT, P, R * C27], DT, kind="ExternalInput")
    t2_d = nc.dram_tensor("t2", [NT, P, R * C27], DT, kind="ExternalInput")
    o1_d = nc.dram_tensor("o1", [NT, P, R * 81], DT, kind="ExternalOutput")
    o2_d = nc.dram_tensor("o2", [NT, P, R * 9], DT, kind="ExternalOutput")
    o3_d = nc.dram_tensor("o3", [NT, P, R], DT, kind="ExternalOutput")

    ident_dram = nc.inline_tensor(
        (_C1 * np.eye(P)).astype(np.float32), name="ident"
    )

    with tile.TileContext(nc) as tc:
        with tc.tile_pool(name="pool", bufs=1) as pool, tc.tile_pool(
            name="psum", bufs=2, space="PSUM"
        ) as psum_pool:
            ident = pool.tile([P, P], DT, name="ident", bufs=1)
            nc.sync.dma_start(out=ident, in_=ident_dram.ap())

            for t in range(NT):
                t1 = pool.tile([P, R * C27], DT, name="t1t", bufs=4)
                t2 = pool.tile([P, R * C27], DT, name="t2t", bufs=4)
                nc.sync.dma_start(out=t1, in_=t1_d[t])
                nc.sync.dma_start(out=t2, in_=t2_d[t])

                t1v = t1.rearrange("p (r c) -> p r c", c=C27)
                t2v = t2.rearrange("p (r c) -> p r c", c=C27)

                # ---- out1: DVE broadcast mults -> PE accumulate -> ACT drain
                pb = pool.tile([P, 3, R * 81], DT, name="pb", bufs=2)
                for k in range(3):
                    a = t1v.rearrange("p r (x k) -> p r x k", k=3)[:, :, :, k]
                    a = a.unsqueeze(3).broadcast_to([P, R, 9, 9])
                    b = t2v.rearrange("p r (k u) -> p r k u", k=3)[:, :, k, :]
                    b = b.unsqueeze(2).broadcast_to([P, R, 9, 9])
                    nc.vector.tensor_mul(
                        pb[:, k].rearrange("p (r x u) -> p r x u", x=9, u=9), a, b
                    )
                o1 = pool.tile([P, R * 81], DT, name="o1", bufs=3)
                for h in range(2):
                    acc = psum_pool.tile([P, 3, 512], DT, name="acc")
                    for c in range(3):
                        lo = h * HALF + c * CHUNK
                        for k in range(3):
                            nc.tensor.matmul(
                                acc[:, c, :CHUNK],
                                ident,
                                pb[:, k, lo : lo + CHUNK],
                                start=(k == 0),
                                stop=(k == 2),
                            )
                    nc.scalar.copy(
                        o1[:, h * HALF : (h + 1) * HALF], acc[:, :, :CHUNK]
                    )

                # ---- out2 on DVE (x-split mults + add tree) ----
                # qq[r, x, u, m] = t1b[r, 9x+m] * t2[r, 3m+u]
                qq = pool.tile([P, R * 81], DT, name="qq", bufs=2)
                qqx = qq.rearrange("p (r x u m) -> p r x u m", x=3, u=3, m=9)
                for x in range(3):
                    a2 = t1v.rearrange("p r (x m) -> p r x m", x=3)[:, :, x, :]
                    a2 = a2.unsqueeze(2).broadcast_to([P, R, 3, 9])  # (R, u, m)
                    b2 = t2v.rearrange("p r (m u) -> p r m u", u=3)
                    b2 = b2.transpose([0, 1, 3, 2])  # (R, u, m) strides (27,1,3)
                    nc.vector.tensor_mul(qqx[:, :, x, :, :], a2, b2)
                qqv = qq.rearrange("p (r xu m) -> p r xu m", xu=9, m=9)
                o2r = pool.tile([P, R * 9], DT, name="o2r", bufs=2)
                nc.vector.tensor_reduce(
                    o2r.rearrange("p (r xu) -> p r xu", xu=9),
                    qqv,
                    mybir.AxisListType.X,
                    add,
                )
                o2 = pool.tile([P, R * 9], DT, name="o2", bufs=3)
                nc.scalar.mul(o2, o2r, _C2)

                # -------- out3: DVE mult + DVE reduce + ACT scale --------
                e = pool.tile([P, R * C27], DT, name="e", bufs=2)
                nc.vector.tensor_mul(e, t1, t2)
                o3r = pool.tile([P, R], DT, name="o3r", bufs=2)
                nc.vector.tensor_reduce(
                    o3r,
                    e.rearrange("p (r c) -> p r c", c=C27),
                    mybir.AxisListType.X,
                    add,
                )
                o3 = pool.tile([P, R], DT, name="o3", bufs=3)
                nc.scalar.mul(o3, o3r, _C3)

                nc.sync.dma_start(out=o1_d[t], in_=o1)
                nc.sync.dma_start(out=o2_d[t], in_=o2)
                nc.sync.dma_start(out=o3_d[t], in_=o3)

    nc.compile()
    return nc


def _get_nc():
    if "nc" not in _CACHE:
        _ensure_paths()
        _CACHE["nc"] = _build_nc()
    return _CACHE["nc"]


def kernel(T1, T2, _trace=False):
    _ensure_paths()
    from concourse.bass_utils import run_bass_kernel_spmd

    nc = _get_nc()

    T1f = np.ascontiguousarray(np.asarray(T1, dtype=np.float32)).reshape(ROWS, C27)
    T2f = np.ascontiguousarray(np.asarray(T2, dtype=np.float32)).reshape(ROWS, C27)

    in_maps = []
    for i in range(N_CORES):
        sl = slice(i * RPC, (i + 1) * RPC)
        in_maps.append(
            {
                "t1": np.ascontiguousarray(T1f[sl]).reshape(NT, P, R * C27),
                "t2": np.ascontiguousarray(T2f[sl]).reshape(NT, P, R * C27),
            }
        )

    res = run_bass_kernel_spmd(nc, in_maps, list(range(N_CORES)), trace=_trace)
    _CACHE["last_results"] = res

    out1 = np.empty((ROWS, 81), dtype=np.float32)
    out2 = np.empty((ROWS, 9), dtype=np.float32)
    out3 = np.empty((ROWS,), dtype=np.float32)
    for i in range(N_CORES):
        sl = slice(i * RPC, (i + 1) * RPC)
        r = res.results[i]
        out1[sl] = r["o1"].reshape(RPC, 81)
        out2[sl] = r["o2"].reshape(RPC, 9)
        out3[sl] = r["o3"].reshape(RPC)

    return (
        out1.reshape(B, C, 3, 3, 3, 3),
        out2.reshape(B, C, 3, 3),
        out3.reshape(B, C),
    )


# revision 13
# speedup vs baseline: 1.0473x; 1.0028x over previous
"""Trainium2 Bass kernel for nn_BroadcastTC.

Math (per (b, c) pair; T1/T2 are 27-element tensors viewed as 3x3x3):
  out1[b,c,x,y,u,v] = sum_k T1[b,c,x,y,k] * T2[b,c,k,u,v] / sqrt(3)
  out2[b,c,x,u]     = sum_{k,l} T1[b,c,x,k,l] * T2[b,c,k,l,u] / 3
  out3[b,c]         = sum_{k,l,m} T1[b,c,k,l,m] * T2[b,c,k,l,m] / sqrt(27)

Pure data parallel over 8 NeuronCores (batch axis). Each core processes its
131072 (b,c)-rows in tiles of [128 partitions x R rows].

v3 engine split (DVE and GPSIMD serialize on the shared SBUF port, so GPSIMD
is retired; the independent-port engines PE/ACT take the accumulations):
  - VectorE:  all elementwise multiplies (out1: 3 broadcast mults; out2:
              3 x-split mults; out3: dense mult) + out2 add-tree + out3 reduce
  - TensorE:  out1 = P0+P1+P2 via exact fp32 identity matmuls accumulated
              in PSUM (bank-aligned 432-col chunks)
  - ScalarE:  T1 pre-scales (normalizers fold in), PSUM->SBUF drain of out1,
              out3 post-scale
  - All DMA on HWDGE (nc.sync).
"""

import sys

import numpy as np

B, C = 8192, 128
N_CORES = 8
P = 128            # SBUF partitions
R = 32             # rows per partition per tile
C27 = 27
ROWS = B * C                      # 1048576 total (b,c) rows
RPC = ROWS // N_CORES             # 131072 rows per core
TILE_ROWS = P * R                 # 4096 rows per tile
NT = RPC // TILE_ROWS             # 32 tiles per core

HALF = R * 81 // 2                # 1296 out1 cols per half-tile
CHUNK = HALF // 3                 # 432 cols per PSUM bank (<=512)

_C1 = float(1.0 / np.sqrt(3.0))   # out1 normalizer (folded into PE weights)
_C2 = float(1.0 / 3.0)            # out2 normalizer (ACT post-scale)
_C3 = float(1.0 / np.sqrt(27.0))  # out3 normalizer (ACT post-scale)

_CACHE = {}


def _ensure_paths():
    for p in ("/opt/trn_rl_repo", "/root/.axon_site/_ro/trn_rl_repo"):
        try:
            import concourse  # noqa: F401
            return
        except ImportError:
            if p not in sys.path:
                sys.path.insert(0, p)
    import concourse  # noqa: F401


def _build_nc():
    import concourse.bacc as bacc
    import concourse.tile as tile
    from concourse import mybir

    DT = mybir.dt.float32
    add = mybir.AluOpType.add

    nc = bacc.Bacc("TRN2", target_bir_lowering=False, debug=False)

    t1_d = nc.dram_tensor("t1", [NT, P, R * C27], DT, kind="ExternalInput")
    t2_d = nc.dram_tensor("t2", [NT, P, R * C27], DT, kind="ExternalInput")
    o1_d = nc.dram_tensor("o1", [NT, P, R * 81], DT, kind="ExternalOutput")
    o2_d = nc.dram_tensor("o2", [NT, P, R * 9], DT, kind="ExternalOutput")
    o3_d = nc.dram_tensor("o3", [NT, P, R], DT, kind="ExternalOutput")

    ident_dram = nc.inline_tensor(
        (_C1 * np.eye(P)).astype(np.float32), name="ident"
    )

    with tile.TileContext(nc) as tc:
        with tc.tile_pool(name="pool", bufs=1) as pool, tc.tile_pool(
            name="psum", bufs=2, space="PSUM"
        ) as psum_pool:
            ident = pool.tile([P, P], DT, name="ident", bufs=1)
            nc.sync.dma_start(out=ident, in_=ident_dram.ap())

            # sub-tile schedule: quarter-size edge tiles for faster
            # pipeline fill/drain; full tiles in the middle
            RQ = R // 4
            sched = [(0, r0, RQ) for r0 in range(0, R, RQ)]
            sched += [(t, 0, R) for t in range(1, NT - 1)]
            sched += [(NT - 1, r0, RQ) for r0 in range(0, R, RQ)]

            for td, r0, Rt in sched:
                half_t = Rt * 81 // 2
                chunk_t = half_t // 3
                t1 = pool.tile([P, Rt * C27], DT, name="t1t", bufs=4)
                t2 = pool.tile([P, Rt * C27], DT, name="t2t", bufs=4)
                nc.sync.dma_start(
                    out=t1, in_=t1_d[td][:, r0 * C27 : (r0 + Rt) * C27]
                )
                nc.sync.dma_start(
                    out=t2, in_=t2_d[td][:, r0 * C27 : (r0 + Rt) * C27]
                )

                t1v = t1.rearrange("p (r c) -> p r c", c=C27)
                t2v = t2.rearrange("p (r c) -> p r c", c=C27)

                # ---- out1: DVE broadcast mults -> PE accumulate -> ACT drain
                pb = pool.tile([P, 3, Rt * 81], DT, name="pb", bufs=2)
                for k in range(3):
                    a = t1v.rearrange("p r (x k) -> p r x k", k=3)[:, :, :, k]
                    a = a.unsqueeze(3).broadcast_to([P, Rt, 9, 9])
                    b = t2v.rearrange("p r (k u) -> p r k u", k=3)[:, :, k, :]
                    b = b.unsqueeze(2).broadcast_to([P, Rt, 9, 9])
                    nc.vector.tensor_mul(
                        pb[:, k].rearrange("p (r x u) -> p r x u", x=9, u=9), a, b
                    )
                o1 = pool.tile([P, Rt * 81], DT, name="o1", bufs=3)
                for h in range(2):
                    acc = psum_pool.tile([P, 3, 512], DT, name="acc")
                    for c in range(3):
                        lo = h * half_t + c * chunk_t
                        for k in range(3):
                            nc.tensor.matmul(
                                acc[:, c, :chunk_t],
                                ident,
                                pb[:, k, lo : lo + chunk_t],
                                start=(k == 0),
                                stop=(k == 2),
                            )
                    nc.scalar.copy(
                        o1[:, h * half_t : (h + 1) * half_t], acc[:, :, :chunk_t]
                    )

                # ---- out2 on DVE (x-split mults + m-reduce) ----
                # qq[r, x, u, m] = t1[r, 9x+m] * t2[r, 3m+u]
                qq = pool.tile([P, Rt * 81], DT, name="qq", bufs=2)
                qqx = qq.rearrange("p (r x u m) -> p r x u m", x=3, u=3, m=9)
                for x in range(3):
                    a2 = t1v.rearrange("p r (x m) -> p r x m", x=3)[:, :, x, :]
                    a2 = a2.unsqueeze(2).broadcast_to([P, Rt, 3, 9])  # (R, u, m)
                    b2 = t2v.rearrange("p r (m u) -> p r m u", u=3)
                    b2 = b2.transpose([0, 1, 3, 2])  # (R, u, m) strides (27,1,3)
                    nc.vector.tensor_mul(qqx[:, :, x, :, :], a2, b2)
                qqv = qq.rearrange("p (r xu m) -> p r xu m", xu=9, m=9)
                o2r = pool.tile([P, Rt * 9], DT, name="o2r", bufs=2)
                nc.vector.tensor_reduce(
                    o2r.rearrange("p (r xu) -> p r xu", xu=9),
                    qqv,
                    mybir.AxisListType.X,
                    add,
                )
                o2 = pool.tile([P, Rt * 9], DT, name="o2", bufs=3)
                nc.scalar.mul(o2, o2r, _C2)

                # -------- out3: DVE mult + DVE reduce + ACT scale --------
                e = pool.tile([P, Rt * C27], DT, name="e", bufs=2)
                nc.vector.tensor_mul(e, t1, t2)
                o3r = pool.tile([P, Rt], DT, name="o3r", bufs=2)
                nc.vector.tensor_reduce(
                    o3r,
                    e.rearrange("p (r c) -> p r c", c=C27),
                    mybir.AxisListType.X,
                    add,
                )
                o3 = pool.tile([P, Rt], DT, name="o3", bufs=3)
                nc.scalar.mul(o3, o3r, _C3)

                nc.sync.dma_start(
                    out=o1_d[td][:, r0 * 81 : (r0 + Rt) * 81], in_=o1
                )
                nc.sync.dma_start(
                    out=o2_d[td][:, r0 * 9 : (r0 + Rt) * 9], in_=o2
                )
                nc.sync.dma_start(out=o3_d[td][:, r0 : r0 + Rt], in_=o3)

    nc.compile()
    return nc


def _get_nc():
    if "nc" not in _CACHE:
        _ensure_paths()
        _CACHE["nc"] = _build_nc()
    return _CACHE["nc"]


def kernel(T1, T2, _trace=False):
    _ensure_paths()
    from concourse.bass_utils import run_bass_kernel_spmd

    nc = _get_nc()

    T1f = np.ascontiguousarray(np.asarray(T1, dtype=np.float32)).reshape(ROWS, C27)
    T2f = np.ascontiguousarray(np.asarray(T2, dtype=np.float32)).reshape(ROWS, C27)

    in_maps = []
    for i in range(N_CORES):
        sl = slice(i * RPC, (i + 1) * RPC)
        in_maps.append(
            {
                "t1": np.ascontiguousarray(T1f[sl]).reshape(NT, P, R * C27),
                "t2": np.ascontiguousarray(T2f[sl]).reshape(NT, P, R * C27),
            }
        )

    res = run_bass_kernel_spmd(nc, in_maps, list(range(N_CORES)), trace=_trace)
    _CACHE["last_results"] = res

    out1 = np.empty((ROWS, 81), dtype=np.float32)
    out2 = np.empty((ROWS, 9), dtype=np.float32)
    out3 = np.empty((ROWS,), dtype=np.float32)
    for i in range(N_CORES):
        sl = slice(i * RPC, (i + 1) * RPC)
        r = res.results[i]
        out1[sl] = r["o1"].reshape(RPC, 81)
        out2[sl] = r["o2"].reshape(RPC, 9)
        out3[sl] = r["o3"].reshape(RPC)

    return (
        out1.reshape(B, C, 3, 3, 3, 3),
        out2.reshape(B, C, 3, 3),
        out3.reshape(B, C),
    )
